# revision 15
# baseline (speedup 1.0000x reference)
"""Trainium2 Bass kernel for nn_BakedAttentionHead.

Reference computation (per row b of query):
    s      = (q @ K^T) / sqrt(D)                      # (B, N)
    e'     = exp(s - max_n s)
    d      = 1 + sum_n e'
    recip  = 16-step sigmoid long-division approx of 1/d
    out    = (e' * recip) @ V

Kernel restructuring (algebraically equivalent, fp-wise ~1e-7 of reference):
    e      = exp(s)                 (raw; |s| <= ~6 so no overflow)
    em     = exp(-max_n s)
    d      = 1 + (sum_n e) * em
    out    = (e @ V) * (em * recip) per row    (row scale folded into PSUM evac)

Sharding: data-parallel over the 8192 query rows -> 8 cores x 1024 rows.
keys/values replicated. Matmuls run in float32r (full-rate fp32 PE mode).
mm1 is computed in transposed orientation (scores^T: [n, m]) so that the
exp'd tiles are directly the lhsT operand of mm2 with no transposes of the
big intermediate; only the tiny [128, 256] max/sum stat tensors go through
PE transposes for the cross-partition reduction.
"""

import numpy as np

B, D, N = 8192, 1024, 2048
NCORES = 8
M = B // NCORES            # 1024 query rows per core
MB = 4                     # m blocks per core
MBS = M // MB              # 256 m per block
MSUB = MBS // 128          # 2 psum-row subtiles per block
NT = N // 128              # 16 n tiles
DT = D // 128              # 8 d (contraction) tiles
DO = 2                     # output dout chunks of 512
SCALE = 0.03125            # D ** -0.5
SIG_SCALE = 100.0
BITS = 16

_CACHE = {}


def _build(ps1_bufs=3, ps2_bufs=3, pst_bufs=2, e_bufs=2 * NT, qt_bufs=2,
           order="pair", skip_mm2=False, skip_scan=False, skip_stats=False):
    import concourse.mybir as mybir
    import concourse.tile as tile
    from concourse import bacc
    from concourse.masks import make_identity

    F32 = mybir.dt.float32
    F32R = mybir.dt.float32r
    AX = mybir.AxisListType
    OP = mybir.AluOpType
    AF = mybir.ActivationFunctionType

    nc = bacc.Bacc("TRN2", target_bir_lowering=False, debug=False,
                   num_devices=NCORES)
    qT_d = nc.declare_dram_parameter("qT", [D, M], F32R, isOutput=False)
    kT_d = nc.declare_dram_parameter("kT", [D, N], F32R, isOutput=False)
    v_d = nc.declare_dram_parameter("v", [N, D], F32R, isOutput=False)
    out_d = nc.declare_dram_parameter("out", [M, D], F32, isOutput=True)

    qT_ap = qT_d[:].rearrange("(dt p) m -> p dt m", p=128)
    kT_ap = kT_d[:].rearrange("(dt p) n -> p dt n", p=128)
    v_ap = v_d[:].rearrange("(nt p) do -> p nt do", p=128)

    with tile.TileContext(nc) as tc:
        with (
            tc.tile_pool(name="res", bufs=1) as res_pool,
            tc.tile_pool(name="qt", bufs=qt_bufs) as qt_pool,
            tc.tile_pool(name="e", bufs=e_bufs) as e_pool,
            tc.tile_pool(name="acc", bufs=2) as acc_pool,
            tc.tile_pool(name="stat", bufs=2) as stat_pool,
            tc.tile_pool(name="o", bufs=5) as out_pool,
            tc.tile_pool(name="ps1", bufs=ps1_bufs, space="PSUM") as ps1_pool,
            tc.tile_pool(name="ps2", bufs=ps2_bufs, space="PSUM") as ps2_pool,
            tc.tile_pool(name="pst", bufs=pst_bufs, space="PSUM") as pst_pool,
        ):
            # Load order matters: the SP HWDGE queue is FIFO, so emit the
            # chunks gating the first matmuls first.  vt rides the scalar
            # engine's separate HWDGE ring in parallel; outputs go via
            # gpsimd SWDGE so they never sit behind input loads.
            kt = res_pool.tile([128, DT, N], F32R)
            qt = res_pool.tile([128, DT, M], F32R)
            nc.sync.dma_start(out=kt[:, :, 0:512], in_=kT_ap[:, :, 0:512])
            nc.sync.dma_start(out=qt[:, :, 0:MBS], in_=qT_ap[:, :, 0:MBS])
            for c in range(1, 4):
                nc.sync.dma_start(out=kt[:, :, c * 512:(c + 1) * 512],
                                  in_=kT_ap[:, :, c * 512:(c + 1) * 512])
            for c in range(1, MB):
                nc.sync.dma_start(out=qt[:, :, c * MBS:(c + 1) * MBS],
                                  in_=qT_ap[:, :, c * MBS:(c + 1) * MBS])
            vt = res_pool.tile([128, NT, D], F32R)
            for c in range(4):
                nc.scalar.dma_start(out=vt[:, c * 4:(c + 1) * 4, :],
                                    in_=v_ap[:, c * 4:(c + 1) * 4, :])
            ident = res_pool.tile([128, 128], F32)
            make_identity(nc, ident[:])

            pair_state = {}

            def emit_mm1(mb):
                """scores^T for block mb; exp into e tiles; max/sum acc; stats."""
                etiles = []
                macc = acc_pool.tile([128, MBS], F32, name=f"macc{mb}", tag="macc")
                sacc = acc_pool.tile([128, MBS], F32, name=f"sacc{mb}", tag="sacc")
                for nt in range(NT):
                    ps = ps1_pool.tile([128, MBS], F32, name=f"s{mb}_{nt}", tag="ps1")
                    for dt in range(DT):
                        nc.tensor.matmul(
                            ps[:],
                            lhsT=kt[:, dt, nt * 128:(nt + 1) * 128],
                            rhs=qt[:, dt, mb * MBS:(mb + 1) * MBS],
                            start=(dt == 0), stop=(dt == DT - 1),
                        )
                    e_nt = e_pool.tile([128, MBS], F32R, name=f"e{mb}_{nt}", tag="e")
                    nc.scalar.activation(e_nt[:], ps[:], AF.Exp, scale=SCALE)
                    if nt == 0:
                        nc.vector.tensor_copy(macc[:], ps[:])
                        nc.vector.tensor_copy(sacc[:], e_nt[:].bitcast(F32))
                    else:
                        nc.vector.tensor_tensor(
                            out=macc[:], in0=ps[:], in1=macc[:], op=OP.max)
                        nc.vector.tensor_tensor(
                            out=sacc[:], in0=e_nt[:].bitcast(F32), in1=sacc[:],
                            op=OP.add)
                    etiles.append(e_nt)

                if skip_stats:
                    return etiles
                # cross-partition max/sum via PE transpose + free-dim reduce
                pair = pair_state[mb // 2]
                slot = (mb % 2) * MSUB
                mx_b = stat_pool.tile([128, MSUB], F32, name=f"mx{mb}", tag="mx")
                s_b = stat_pool.tile([128, MSUB], F32, name=f"s{mb}", tag="s")
                for c in range(MSUB):
                    pt = pst_pool.tile([128, 128], F32, name=f"tm{mb}_{c}", tag="pst")
                    nc.tensor.transpose(
                        pt[:], macc[:, c * 128:(c + 1) * 128], ident[:])
                    nc.vector.tensor_reduce(
                        mx_b[:, c:c + 1], pt[:], axis=AX.X, op=OP.max)
                    pt2 = pst_pool.tile([128, 128], F32, name=f"ts{mb}_{c}", tag="pst")
                    nc.tensor.transpose(
                        pt2[:], sacc[:, c * 128:(c + 1) * 128], ident[:])
                    nc.vector.tensor_reduce(
                        s_b[:, c:c + 1], pt2[:], axis=AX.X, op=OP.add)
                # em = exp(-scale * mx); d = 1 + s * em
                empair, dpair = pair["em"], pair["d"]
                nc.scalar.activation(
                    empair[:, slot:slot + MSUB], mx_b[:], AF.Exp, scale=-SCALE)
                tmp = stat_pool.tile([128, MSUB], F32, name=f"dt{mb}", tag="dtmp")
                nc.vector.tensor_tensor(
                    out=tmp[:], in0=s_b[:], in1=empair[:, slot:slot + MSUB],
                    op=OP.mult)
                nc.vector.tensor_scalar_add(dpair[:, slot:slot + MSUB], tmp[:], 1.0)
                return etiles

            def emit_scan(pair):
                """16-step sigmoid long-division on d: [128, 4] batched."""
                d_t = pair["d"]
                w = 2 * MSUB
                r0 = stat_pool.tile([128, w], F32, name="r0", tag="r0")
                r1 = stat_pool.tile([128, w], F32, name="r1", tag="r1")
                q0 = stat_pool.tile([128, w], F32, name="q0", tag="q0")
                q1 = stat_pool.tile([128, w], F32, name="q1", tag="q1")
                z = stat_pool.tile([128, w], F32, name="z", tag="z")
                st = stat_pool.tile([128, w], F32, name="st", tag="st")
                t = stat_pool.tile([128, w], F32, name="t", tag="t")
                nc.vector.memset(r0[:], 1.0)
                nc.vector.memset(q0[:], 0.0)
                r, q = r0, r1
                qa, qb = q0, q1
                for i in range(BITS):
                    rn = r1 if r is r0 else r0
                    qn = q1 if qa is q0 else q0
                    # z = 2r - d
                    nc.vector.scalar_tensor_tensor(
                        out=z[:], in0=r[:], scalar=2.0, in1=d_t[:],
                        op0=OP.mult, op1=OP.subtract)
                    # step = sigmoid(100 z)
                    nc.scalar.activation(st[:], z[:], AF.Sigmoid, scale=SIG_SCALE)
                    # t = d * step
                    nc.vector.tensor_tensor(out=t[:], in0=d_t[:], in1=st[:],
                                            op=OP.mult)
                    # r' = 2r - t
                    nc.vector.scalar_tensor_tensor(
                        out=rn[:], in0=r[:], scalar=2.0, in1=t[:],
                        op0=OP.mult, op1=OP.subtract)
                    # q' = step * w_i + q
                    nc.vector.scalar_tensor_tensor(
                        out=qn[:], in0=st[:], scalar=float(2.0 ** -(i + 1)),
                        in1=qa[:], op0=OP.mult, op1=OP.add)
                    r, qa = rn, qn
                # final row scale = em * q
                nc.vector.tensor_tensor(out=pair["scale"][:], in0=pair["em"][:],
                                        in1=qa[:], op=OP.mult)

            def emit_mm2(mb, etiles):
                if skip_mm2:
                    # consume e tiles so pools release (cheap copy of one tile)
                    return
                pair = pair_state[mb // 2]
                slot = (mb % 2) * MSUB
                for do in range(DO):
                    for c in range(MSUB):
                        ps = ps2_pool.tile([128, 512], F32,
                                           name=f"o{mb}_{do}_{c}", tag="ps2")
                        for nt in range(NT):
                            nc.tensor.matmul(
                                ps[:],
                                lhsT=etiles[nt][:, c * 128:(c + 1) * 128],
                                rhs=vt[:, nt, do * 512:(do + 1) * 512],
                                start=(nt == 0), stop=(nt == NT - 1),
                            )
                        ot = out_pool.tile([128, 512], F32,
                                           name=f"ot{mb}_{do}_{c}", tag="ot")
                        # plain evac frees the PSUM bank without waiting on
                        # the reciprocal scan; the row scale is applied in a
                        # separate DVE pass right before the store
                        nc.scalar.activation(ot[:], ps[:], AF.Copy)
                        nc.vector.tensor_scalar_mul(
                            ot[:], ot[:],
                            pair["scale"][:, slot + c:slot + c + 1])
                        m0 = mb * MBS + c * 128
                        nc.gpsimd.dma_start(
                            out=out_d[m0:m0 + 128, do * 512:(do + 1) * 512],
                            in_=ot[:])

            for p in range(MB // 2):
                pair_state[p] = {
                    "em": stat_pool.tile([128, 2 * MSUB], F32,
                                         name=f"em{p}", tag="em"),
                    "d": stat_pool.tile([128, 2 * MSUB], F32,
                                        name=f"d{p}", tag="d"),
                    "scale": stat_pool.tile([128, 2 * MSUB], F32,
                                            name=f"sc{p}", tag="sc"),
                }
                if order == "pair":
                    e_lo = emit_mm1(2 * p)
                    e_hi = emit_mm1(2 * p + 1)
                    if not (skip_scan or skip_stats):
                        emit_scan(pair_state[p])
                    emit_mm2(2 * p, e_lo)
                    emit_mm2(2 * p + 1, e_hi)
                else:  # interleave mm2(lo) before mm1(hi)
                    e_lo = emit_mm1(2 * p)
                    emit_scan_lo = None
                    e_hi = emit_mm1(2 * p + 1)
                    emit_scan(pair_state[p])
                    emit_mm2(2 * p, e_lo)
                    emit_mm2(2 * p + 1, e_hi)

    nc.compile()
    return nc


def _get_nc():
    if "nc" not in _CACHE:
        _CACHE["nc"] = _build()
    return _CACHE["nc"]


def kernel(query, keys, values):
    from concourse.bass_utils import run_bass_kernel_spmd

    query = np.ascontiguousarray(query, dtype=np.float32)
    keys = np.ascontiguousarray(keys, dtype=np.float32)
    values = np.ascontiguousarray(values, dtype=np.float32)

    nc = _get_nc()
    kT = np.ascontiguousarray(keys.T)
    in_maps = []
    for i in range(NCORES):
        qT = np.ascontiguousarray(query[i * M:(i + 1) * M].T)
        in_maps.append({"qT": qT, "kT": kT, "v": values})
    res = run_bass_kernel_spmd(nc, in_maps, list(range(NCORES)))
    out = np.concatenate([res.results[i]["out"] for i in range(NCORES)], axis=0)
    return np.ascontiguousarray(out, dtype=np.float32)
